# revision 1
# baseline (speedup 1.0000x reference)
import numpy as np
import jax
import jax.numpy as jnp
from functools import partial

# GPT-MoD dims (hardcoded per problem spec)
B, T, V, C, H, L = 4, 1024, 50257, 768, 6, 6
HS = C // H
NEG = -1e30
NDEV = 8
VP = ((V + NDEV - 1) // NDEV) * NDEV   # 50264, vocab padded to 8 shards
VS = VP // NDEV


def _ln(x, g, b):
    m = x.mean(-1, keepdims=True)
    v = x.var(-1, keepdims=True)
    return (x - m) * jax.lax.rsqrt(v + 1e-5) * g + b


@jax.jit
def _body(idx, tok_emb, pos_emb, router_w, router_b, aux_w, aux_b,
          ln1_g, ln1_b, ln2_g, ln2_b, wq, wk, wv, proj_w, proj_b,
          ffn_w1, ffn_b1, ffn_w2, ffn_b2, lnf_g, lnf_b):
    x = tok_emb[idx] + pos_emb[None, :, :]
    tril = jnp.tril(jnp.ones((T, T), bool))

    def layer(x, w):
        (rw_w, rw_b, aw, ab, l1g, l1b, l2g, l2b,
         wq_l, wk_l, wv_l, pw, pb, f1w, f1b, f2w, f2b) = w
        rw = x @ rw_w + rw_b
        sel = (x @ aw + ab) > 0.0
        h = _ln(x, l1g, l1b)
        q = jnp.einsum('btc,hcd->bhtd', h, wq_l)
        k = jnp.einsum('btc,hcd->bhtd', h, wk_l)
        v = jnp.einsum('btc,hcd->bhtd', h, wv_l)
        scores = jnp.einsum('bhtd,bhsd->bhts', q, k) * (HS ** -0.5)
        mask = sel[:, None, :, None] & sel[:, None, None, :] & tril
        wei = jax.nn.softmax(jnp.where(mask, scores, NEG), axis=-1)
        att = jnp.einsum('bhts,bhsd->bhtd', wei, v)
        att = att.transpose(0, 2, 1, 3).reshape(B, T, C)
        y = x + att @ pw + pb
        f = jax.nn.relu(_ln(y, l2g, l2b) @ f1w + f1b) @ f2w + f2b
        blk = y + f
        x = jnp.where(sel[..., None], blk * rw[..., None], x)
        return x, None

    ws = (router_w, router_b, aux_w, aux_b, ln1_g, ln1_b, ln2_g, ln2_b,
          wq, wk, wv, proj_w, proj_b, ffn_w1, ffn_b1, ffn_w2, ffn_b2)
    x, _ = jax.lax.scan(layer, x, ws)
    return _ln(x, lnf_g, lnf_b)


@partial(jax.pmap, in_axes=(None, 0, 0))
def _head(x, w, b):
    return x @ w + b


def kernel(**inputs):
    inputs = {k: np.asarray(v) for k, v in inputs.items()}
    idx = inputs.pop('idx').astype(np.int32)
    lm_w = inputs.pop('lm_w').astype(np.float32)
    lm_b = inputs.pop('lm_b').astype(np.float32)
    rest = {k: np.asarray(v, np.float32) for k, v in inputs.items()}

    x = _body(idx, rest['tok_emb'], rest['pos_emb'],
              rest['router_w'], rest['router_b'], rest['aux_w'], rest['aux_b'],
              rest['ln1_g'], rest['ln1_b'], rest['ln2_g'], rest['ln2_b'],
              rest['wq'], rest['wk'], rest['wv'], rest['proj_w'], rest['proj_b'],
              rest['ffn_w1'], rest['ffn_b1'], rest['ffn_w2'], rest['ffn_b2'],
              rest['lnf_g'], rest['lnf_b'])

    wp = np.zeros((C, VP), np.float32)
    wp[:, :V] = lm_w
    bp = np.zeros((VP,), np.float32)
    bp[:V] = lm_b
    wsh = np.ascontiguousarray(wp.reshape(C, NDEV, VS).transpose(1, 0, 2))
    bsh = bp.reshape(NDEV, VS)

    try:
        res = _head(x, wsh, bsh)                    # [8, B, T, VS]
        out = np.asarray(res)
        logits = np.moveaxis(out, 0, 2).reshape(B, T, VP)[:, :, :V]
    except Exception:
        logits = np.asarray(jnp.asarray(x) @ lm_w + lm_b)
    return np.ascontiguousarray(logits)



# revision 4
# speedup vs baseline: 6041.9409x; 6041.9409x over previous
import numpy as np
import jax
import jax.numpy as jnp
from jax.sharding import Mesh, PartitionSpec as P, NamedSharding
try:
    from jax.experimental.shard_map import shard_map
except ImportError:
    from jax.shard_map import shard_map  # newer jax

# GPT-MoD dims (hardcoded per problem spec)
B, T, V, C, H, L = 4, 1024, 50257, 768, 6, 6
HS = C // H                 # 128
FF = 4 * C                  # 3072
NEG = -1e30
NB, NT = 4, 2               # mesh: 4-way batch parallel x 2-way tensor parallel
ND = NB * NT
VP = ((V + ND - 1) // ND) * ND   # 50264
VS = VP // ND                    # 6283
HL = H // NT                     # 3 heads per TP rank
CL = HL * HS                     # 384
FL = FF // NT                    # 1536

_mesh = None


def _get_mesh():
    global _mesh
    if _mesh is None:
        devs = np.array(jax.devices()[:ND]).reshape(NB, NT)
        _mesh = Mesh(devs, ('b', 't'))
    return _mesh


def _ln(x, g, b):
    m = x.mean(-1, keepdims=True)
    v = x.var(-1, keepdims=True)
    return (x - m) * jax.lax.rsqrt(v + 1e-5) * g + b


def _fwd_local(x0, rw_w, rw_b, aw, ab, l1g, l1b, l2g, l2b,
               wqf, wkf, wvf, pw, pb, f1w, f1b, f2w, f2b,
               lnfg, lnfb, lmw, lmb):
    # Body runs entirely in f32: MoD routing thresholds (x @ aux_w > 0) are
    # discontinuous, so the residual stream must track the f32 reference
    # bit-closely or token selections flip and produce O(1) logit errors.
    # Only the lm_head (feeds nothing downstream) runs in bf16.
    # x0: [1,T,C] f32 (local batch shard, replicated over 't')
    # wqf/wkf/wvf: [L,C,CL] f32   pw: [L,CL,C] f32
    # f1w: [L,C,FL] f32  f1b: [L,FL] f32  f2w: [L,FL,C] f32
    # lmw: [C,VS] bf16  lmb: [VS] f32
    f32 = jnp.float32
    bf16 = jnp.bfloat16
    tril = jnp.tril(jnp.ones((T, T), bool))
    scale = HS ** -0.5
    x = x0
    for l in range(L):
        rw = jnp.einsum('btc,c->bt', x, rw_w[l]) + rw_b[l]
        sel = (jnp.einsum('btc,c->bt', x, aw[l]) + ab[l]) > 0.0
        h = _ln(x, l1g[l], l1b[l])[0]                        # [T,C]
        q = jnp.einsum('tc,cd->td', h, wqf[l])
        k = jnp.einsum('tc,cd->td', h, wkf[l])
        v = jnp.einsum('tc,cd->td', h, wvf[l])
        q = q.reshape(T, HL, HS).transpose(1, 0, 2)          # [HL,T,HS]
        k = k.reshape(T, HL, HS).transpose(1, 0, 2)
        v = v.reshape(T, HL, HS).transpose(1, 0, 2)
        scores = jnp.einsum('htd,hsd->hts', q, k) * scale
        bias = jnp.where(tril & sel[0][None, :], 0.0, NEG).astype(f32)  # [T,T] keys masked
        wei = jax.nn.softmax(scores + bias[None], axis=-1)
        att = jnp.einsum('hts,hsd->htd', wei, v)
        attf = att.transpose(1, 0, 2).reshape(T, CL)         # [T,CL]
        y_part = jnp.einsum('tc,cd->td', attf, pw[l])
        y = x + (jax.lax.psum(y_part, 't') + pb[l])[None]
        h2 = _ln(y, l2g[l], l2b[l])[0]
        a1 = jax.nn.relu(jnp.einsum('tc,cf->tf', h2, f1w[l]) + f1b[l])
        f_part = jnp.einsum('tf,fc->tc', a1, f2w[l])
        f = jax.lax.psum(f_part, 't') + f2b[l]
        blk = y + f[None]
        x = jnp.where(sel[..., None], blk * rw[..., None], x)
    xf = _ln(x, lnfg, lnfb).astype(bf16)                    # [1,T,C]
    xg = jax.lax.all_gather(xf, 'b', axis=0, tiled=True)    # [B,T,C]
    logits = jnp.einsum('btc,cv->btv', xg, lmw, preferred_element_type=f32) + lmb
    return logits.astype(jnp.float16)                       # [B,T,VS] local


_run_jit = None


def _get_run():
    global _run_jit
    if _run_jit is None:
        mesh = _get_mesh()
        rep = P()
        in_specs = (
            P('b', None, None),            # x0
            rep, rep, rep, rep,            # router_w/b, aux_w/b
            rep, rep, rep, rep,            # ln1_g/b, ln2_g/b
            P(None, None, 't'),            # wqf
            P(None, None, 't'),            # wkf
            P(None, None, 't'),            # wvf
            P(None, 't', None),            # pw
            rep,                           # pb
            P(None, None, 't'),            # f1w
            P(None, 't'),                  # f1b
            P(None, 't', None),            # f2w
            rep,                           # f2b
            rep, rep,                      # lnf_g/b
            P(None, ('b', 't')),           # lmw
            P(('b', 't'),),                # lmb
        )
        out_specs = P(None, None, ('b', 't'))
        _run_jit = jax.jit(shard_map(
            _fwd_local, mesh=mesh, in_specs=in_specs, out_specs=out_specs,
            check_rep=False))
    return _run_jit


def prepare(inputs):
    """Host-side preprocessing + staging onto the 8 cores. Returns arg tuple."""
    bf16 = jnp.bfloat16
    inp = {k: np.asarray(v) for k, v in inputs.items()}
    idx = inp['idx'].astype(np.int64)
    # embedding gather on host: avoids shipping the 154MB table over the tunnel
    x0 = inp['tok_emb'][idx].astype(np.float32) + inp['pos_emb'][None].astype(np.float32)

    def flat_qkv(w):  # [L,H,C,HS] -> [L,C,H*HS] with col = h*HS+d
        return np.ascontiguousarray(w.transpose(0, 2, 1, 3).reshape(L, C, H * HS))

    lm_w = np.zeros((C, VP), np.float32)
    lm_w[:, :V] = inp['lm_w']
    lm_b = np.zeros((VP,), np.float32)
    lm_b[:V] = inp['lm_b']

    host_args = (
        x0,
        inp['router_w'].astype(np.float32), inp['router_b'].astype(np.float32),
        inp['aux_w'].astype(np.float32), inp['aux_b'].astype(np.float32),
        inp['ln1_g'].astype(np.float32), inp['ln1_b'].astype(np.float32),
        inp['ln2_g'].astype(np.float32), inp['ln2_b'].astype(np.float32),
        flat_qkv(inp['wq']), flat_qkv(inp['wk']), flat_qkv(inp['wv']),
        inp['proj_w'].astype(np.float32), inp['proj_b'].astype(np.float32),
        inp['ffn_w1'].astype(np.float32), inp['ffn_b1'].astype(np.float32),
        inp['ffn_w2'].astype(np.float32), inp['ffn_b2'].astype(np.float32),
        inp['lnf_g'].astype(np.float32), inp['lnf_b'].astype(np.float32),
        lm_w, lm_b,
    )
    bf16_idx = {20}                          # lm_w only; body weights stay f32
    mesh = _get_mesh()
    rep = P()
    specs = (
        P('b', None, None),
        rep, rep, rep, rep,
        rep, rep, rep, rep,
        P(None, None, 't'), P(None, None, 't'), P(None, None, 't'),
        P(None, 't', None), rep,
        P(None, None, 't'), P(None, 't'),
        P(None, 't', None), rep,
        rep, rep,
        P(None, ('b', 't')), P(('b', 't'),),
    )
    staged = []
    for i, (a, s) in enumerate(zip(host_args, specs)):
        if i in bf16_idx:
            a = a.astype(bf16)
        staged.append(jax.device_put(a, NamedSharding(mesh, s)))
    return tuple(staged)


def run(staged):
    return _get_run()(*staged)


def kernel(**inputs):
    try:
        staged = prepare(inputs)
        out = run(staged)                     # [B,T,VP] f16, vocab-sharded
        logits = np.asarray(out).astype(np.float32)[:, :, :V]
        return np.ascontiguousarray(logits)
    except Exception:
        return _kernel_fallback(**inputs)


# ----- single-device fallback (correctness safety net) -----

def _kernel_fallback(**inputs):
    inp = {k: np.asarray(v) for k, v in inputs.items()}
    idx = jnp.asarray(inp['idx'].astype(np.int32))
    f = jax.jit(_fallback_body)
    x = f(idx, *[jnp.asarray(inp[k].astype(np.float32)) for k in
                 ('tok_emb', 'pos_emb', 'router_w', 'router_b', 'aux_w', 'aux_b',
                  'ln1_g', 'ln1_b', 'ln2_g', 'ln2_b', 'wq', 'wk', 'wv',
                  'proj_w', 'proj_b', 'ffn_w1', 'ffn_b1', 'ffn_w2', 'ffn_b2',
                  'lnf_g', 'lnf_b')])
    logits = np.asarray(jnp.asarray(x) @ inp['lm_w'].astype(np.float32)
                        + inp['lm_b'].astype(np.float32))
    return np.ascontiguousarray(logits)


def _fallback_body(idx, tok_emb, pos_emb, router_w, router_b, aux_w, aux_b,
                   ln1_g, ln1_b, ln2_g, ln2_b, wq, wk, wv, proj_w, proj_b,
                   ffn_w1, ffn_b1, ffn_w2, ffn_b2, lnf_g, lnf_b):
    x = tok_emb[idx] + pos_emb[None, :, :]
    tril = jnp.tril(jnp.ones((T, T), bool))

    def layer(x, w):
        (rw_w, rw_b, aw, ab, l1g, l1b, l2g, l2b,
         wq_l, wk_l, wv_l, pw, pb, f1w, f1b, f2w, f2b) = w
        rw = x @ rw_w + rw_b
        sel = (x @ aw + ab) > 0.0
        h = _ln(x, l1g, l1b)
        q = jnp.einsum('btc,hcd->bhtd', h, wq_l)
        k = jnp.einsum('btc,hcd->bhtd', h, wk_l)
        v = jnp.einsum('btc,hcd->bhtd', h, wv_l)
        scores = jnp.einsum('bhtd,bhsd->bhts', q, k) * (HS ** -0.5)
        mask = sel[:, None, :, None] & sel[:, None, None, :] & tril
        wei = jax.nn.softmax(jnp.where(mask, scores, NEG), axis=-1)
        att = jnp.einsum('bhts,bhsd->bhtd', wei, v)
        att = att.transpose(0, 2, 1, 3).reshape(B, T, C)
        y = x + att @ pw + pb
        f = jax.nn.relu(_ln(y, l2g, l2b) @ f1w + f1b) @ f2w + f2b
        blk = y + f
        return jnp.where(sel[..., None], blk * rw[..., None], x), None

    ws = (router_w, router_b, aux_w, aux_b, ln1_g, ln1_b, ln2_g, ln2_b,
          wq, wk, wv, proj_w, proj_b, ffn_w1, ffn_b1, ffn_w2, ffn_b2)
    x, _ = jax.lax.scan(layer, x, ws)
    return _ln(x, lnf_g, lnf_b)
